# revision 35
# baseline (speedup 1.0000x reference)
"""Event-RGB dynamic fusion module on 8 trn2 NeuronCores.

Per-pixel dynamic 3x3 depthwise kernels predicted from concat(rgb, event)
via two 1x1 convs + relu, applied to reflect-padded rgb.

Sharding: 8 shards = (batch b in 0..3) x (H half in {0,1}); each core gets
reflect-padded rgb slabs (two bf16 copies at element offsets 0/1 so every
3x3-shift view stays 4-byte aligned for DVE 2x mode), a bf16 event slab,
and replicated pre-laid-out bf16 weights. Fully data-parallel, no
collectives.

Pipeline per 16-row block (rows packed as [128] = 64ch x {top,bottom half}):
  mm1 (PE, K=96 via 64+32 accum)  -> h4 psum, relu+b1 on ACT -> h4 bf16
  mm2 (PE, 9 taps x [K=32,M=64] row/col-group packed) -> dk psum fp32
  apply: (dk + b2) * patch summed over taps, split across DVE/ACT/GPSIMD
"""

import os
from contextlib import ExitStack

import ml_dtypes
import numpy as np

import concourse.bass as bass
import concourse.bacc as bacc
import concourse.mybir as mybir
import concourse.tile as tile
from concourse.bass_utils import run_bass_kernel_spmd

B, C, H, W = 4, 64, 256, 256
CEV, KK, MID = 32, 3, 32
NCORES = 8
SHARD_H = 128          # rows per core
HALF = 64              # rows per half (partition-packing of pixel halves)
RBLK = 16              # rows per half per block
NBLK = HALF // RBLK    # 4
WE = 260               # padded row length (even, so shifted views stay aligned)
SUBR = 4               # rows per half per mm2/apply sub-slice (=1024 px)
NSUB = RBLK // SUBR    # 4
F32 = mybir.dt.float32
BF16 = mybir.dt.bfloat16
AOP = mybir.AluOpType
RELU = mybir.ActivationFunctionType.Relu
IDENT = mybir.ActivationFunctionType.Identity
BF = ml_dtypes.bfloat16

# apply-path assignment per tap ij: "A" = fused STT on DVE (dk read from
# PSUM); "B" = ACT copy+bias to bf16 then mul on GPSIMD; "C" = ACT copy
# then mul on DVE.
PATHS = ["A", "B", "C", "A", "B", "C", "A", "A", "C"]
# engine per tree-add (7 bf16 adds + final fp32 add): "D"=DVE, "G"=GPSIMD
ADD_ENG = ["D", "D", "G", "G", "D", "D", "D", "G"]

_cache = {}


def _build():
    nc = bacc.Bacc("TRN2", target_bir_lowering=False, debug=False)
    rgbe = nc.dram_tensor("rgbe", [C, SHARD_H + 2, WE], BF16, kind="ExternalInput").ap()
    rgbo = nc.dram_tensor("rgbo", [C, SHARD_H + 2, WE], BF16, kind="ExternalInput").ap()
    ev = nc.dram_tensor("ev", [CEV, SHARD_H, W], BF16, kind="ExternalInput").ap()
    w1 = nc.dram_tensor("w1", [128, 384], BF16, kind="ExternalInput").ap()
    w2 = nc.dram_tensor("w2", [128, 384], BF16, kind="ExternalInput").ap()
    bi = nc.dram_tensor("bi", [128, 10], F32, kind="ExternalInput").ap()
    out = nc.dram_tensor("out", [C, SHARD_H, W], F32, kind="ExternalOutput").ap()

    with tile.TileContext(nc) as tc, ExitStack() as ctx:
        _kernel(ctx, tc, rgbe, rgbo, ev, w1, w2, bi, out)
    nc.compile()
    return nc


def _kernel(ctx, tc, rgbe, rgbo, ev, w1, w2, bi, out):
    nc = tc.nc
    consts = ctx.enter_context(tc.tile_pool(name="consts", bufs=1))
    rgb_p = ctx.enter_context(tc.tile_pool(name="rgb", bufs=2))
    ev_p = ctx.enter_context(tc.tile_pool(name="evp", bufs=2))
    h4_p = ctx.enter_context(tc.tile_pool(name="h4", bufs=2))
    dkb_p = ctx.enter_context(tc.tile_pool(name="dkb", bufs=3))
    prod_p = ctx.enter_context(tc.tile_pool(name="prod", bufs=12))
    accp_p = ctx.enter_context(tc.tile_pool(name="accp", bufs=8))
    outt_p = ctx.enter_context(tc.tile_pool(name="outt", bufs=2))
    ph_p = ctx.enter_context(tc.tile_pool(name="psum_h", bufs=2, space="PSUM"))
    pdk_p = ctx.enter_context(tc.tile_pool(name="psum_dk", bufs=3, space="PSUM"))

    w1t = consts.tile([128, 384], BF16)
    nc.sync.dma_start(w1t[:], w1[:])
    w2t = consts.tile([128, 384], BF16)
    nc.sync.dma_start(w2t[:], w2[:])
    bt = consts.tile([128, 10], F32)
    nc.sync.dma_start(bt[:], bi[:])

    npx = RBLK * W           # pixels per half per block (4096)

    for t in range(NBLK):
        rge = rgb_p.tile([128, (RBLK + 2) * WE], BF16, tag="rge")
        nc.sync.dma_start(rge[0:64, :], rgbe[:, t * RBLK:t * RBLK + RBLK + 2, :])
        nc.sync.dma_start(
            rge[64:128, :], rgbe[:, HALF + t * RBLK:HALF + t * RBLK + RBLK + 2, :])
        rgo = rgb_p.tile([128, (RBLK + 2) * WE], BF16, tag="rgo")
        nc.sync.dma_start(rgo[0:64, :], rgbo[:, t * RBLK:t * RBLK + RBLK + 2, :])
        nc.sync.dma_start(
            rgo[64:128, :], rgbo[:, HALF + t * RBLK:HALF + t * RBLK + RBLK + 2, :])
        evt = ev_p.tile([128, RBLK * W], BF16)
        nc.sync.dma_start(evt[64:96, :], ev[:, t * RBLK:t * RBLK + RBLK, :])
        nc.sync.dma_start(
            evt[96:128, :], ev[:, HALF + t * RBLK:HALF + t * RBLK + RBLK, :])

        rgev = rge[:].rearrange("p (r w) -> p r w", w=WE)      # [128, 18, 260]
        rgov = rgo[:].rearrange("p (r w) -> p r w", w=WE)
        evv = evt[:].rearrange("p (r w) -> p r w", w=W)        # [128, 16, 256]

        # ---- mm1: h4[32q+m, px] = relu(b1 + W1 @ concat(rgb, ev)) x4 copies ----
        h4 = h4_p.tile([128, 2 * npx], BF16)
        for s in range(RBLK // 2):               # 512-px slices per half
            r0 = 2 * s
            ph = ph_p.tile([128, 512], F32, tag="ph")
            ph2 = ph_p.tile([128, 512], F32, tag="ph")
            # A-rgb (rows 0-63) and B-rgb (rows 64-127) stream concurrently,
            # then A-ev (64-95) and B-ev (96-127).
            nc.tensor.matmul(ph[:], w1t[0:64, 0:128],
                             rgev[0:64, r0 + 1:r0 + 3, 2:258],
                             start=True, stop=False, tile_position=(0, 0))
            nc.tensor.matmul(ph2[:], w1t[64:128, 128:256],
                             rgev[64:128, r0 + 1:r0 + 3, 2:258],
                             start=True, stop=False, tile_position=(64, 0))
            nc.tensor.matmul(ph[:], w1t[64:96, 0:128],
                             evv[64:96, r0:r0 + 2, :],
                             start=False, stop=True, tile_position=(64, 0))
            nc.tensor.matmul(ph2[:], w1t[96:128, 256:384],
                             evv[96:128, r0:r0 + 2, :],
                             start=False, stop=True, tile_position=(96, 0))
            nc.scalar.activation(h4[:, 512 * s:512 * (s + 1)], ph[:],
                                 RELU, bias=bt[:, 0:1], scale=1.0)
            nc.scalar.activation(h4[:, npx + 512 * s:npx + 512 * (s + 1)], ph2[:],
                                 RELU, bias=bt[:, 0:1], scale=1.0)

        # ---- mm2 + apply, per 4-row sub-slice (1024 px per half) ----
        # taps are processed in groups of 3 (distinct PE row-groups); the 12
        # matmuls of a group are emitted round-robin across the taps so their
        # streams run concurrently in different 32-row strips of the array.
        for s in range(NSUB):
            prods = []
            for g in range(3):
                ijs = [3 * g, 3 * g + 1, 3 * g + 2]
                dks = {ij: pdk_p.tile([128, 1024], F32, name="dk", tag="dk")
                       for ij in ijs}
                for hf in range(2):
                    for nh in range(2):
                        for ij in ijs:
                            rg, slot = ij % 4, ij // 4
                            hc0 = npx * hf + 1024 * s + 512 * nh
                            lh = w2t[32 * rg:32 * rg + 32,
                                     128 * slot + 64 * hf:128 * slot + 64 * hf + 64]
                            nc.tensor.matmul(
                                dks[ij][64 * hf:64 * hf + 64,
                                        512 * nh:512 * nh + 512],
                                lh, h4[32 * rg:32 * rg + 32, hc0:hc0 + 512],
                                start=True, stop=True,
                                tile_position=(32 * rg, 64 * hf))
                for ij in ijs:
                    di, dj = ij // 3 - 1, ij % 3 - 1
                    dk = dks[ij]
                    # patch view: dj=0 from the even-aligned slab, dj=+-1 from
                    # the odd one (keeps every bf16 view 4B-aligned).
                    if dj == 0:
                        src, base = rgev, 2
                    else:
                        src, base = rgov, 1 + dj
                    patch = src[:, SUBR * s + 1 + di:SUBR * s + 5 + di,
                                base:base + 256]
                    dkv = dk[:].rearrange("p (r w) -> p r w", w=W)
                    prod = prod_p.tile([128, 1024], BF16)
                    prodv = prod[:].rearrange("p (r w) -> p r w", w=W)
                    path = PATHS[ij]
                    if path == "A":
                        nc.vector.scalar_tensor_tensor(
                            prodv[:], dkv[:], bt[:, 1 + ij:2 + ij], patch[:],
                            op0=AOP.add, op1=AOP.mult)
                    else:
                        dkb = dkb_p.tile([128, 1024], BF16)
                        nc.scalar.activation(dkb[:], dk[:], IDENT,
                                             bias=bt[:, 1 + ij:2 + ij], scale=1.0)
                        dkbv = dkb[:].rearrange("p (r w) -> p r w", w=W)
                        eng = nc.gpsimd if path == "B" else nc.vector
                        eng.tensor_tensor(prodv[:], dkbv[:], patch[:], op=AOP.mult)
                    prods.append(prod)

            def tadd(i, a, b):
                r = accp_p.tile([128, 1024], BF16, tag="acc", name="acc")
                eng = nc.gpsimd if ADD_ENG[i] == "G" else nc.vector
                eng.tensor_tensor(r[:], a[:], b[:], op=AOP.add)
                return r

            t0 = tadd(0, prods[0], prods[1])
            t1 = tadd(1, prods[2], prods[3])
            t2 = tadd(2, prods[4], prods[5])
            t3 = tadd(3, prods[6], prods[7])
            u0 = tadd(4, t0, t1)
            u1 = tadd(5, t2, t3)
            v = tadd(6, u0, u1)
            ot = outt_p.tile([128, 1024], F32)
            eng = nc.gpsimd if ADD_ENG[7] == "G" else nc.vector
            eng.tensor_tensor(ot[:], v[:], prods[8][:], op=AOP.add)

            otv = ot[:].rearrange("p (r w) -> p r w", w=W)
            ra = t * RBLK + SUBR * s
            nc.sync.dma_start(out[:, ra:ra + SUBR, :], otv[0:64, :, :])
            nc.sync.dma_start(out[:, HALF + ra:HALF + ra + SUBR, :],
                              otv[64:128, :, :])


def _prep_consts(W1, b1, W2, b2):
    W1T = np.ascontiguousarray(W1.T)                              # [96, 32]
    W1T4 = np.tile(W1T, (1, 4))                                   # [96, 128]
    w1sb = np.zeros((128, 384), np.float32)
    w1sb[0:64, 0:128] = W1T4[0:64]          # rgb A
    w1sb[64:96, 0:128] = W1T4[64:96]        # ev A
    w1sb[64:128, 128:256] = W1T4[0:64]      # rgb B
    w1sb[96:128, 256:384] = W1T4[64:96]     # ev B

    W2r = W2.reshape(C, 9, MID)
    w2sb = np.zeros((128, 384), np.float32)
    for ij in range(9):
        rg, slot = ij % 4, ij // 4
        wij = np.ascontiguousarray(W2r[:, ij, :].T)               # [32, 64]
        w2sb[32 * rg:32 * rg + 32, 128 * slot:128 * slot + 64] = wij
        w2sb[32 * rg:32 * rg + 32, 128 * slot + 64:128 * slot + 128] = wij

    bisb = np.zeros((128, 10), np.float32)
    bisb[:, 0] = np.tile(b1, 4)
    b2r = b2.reshape(C, 9)
    for ij in range(9):
        bisb[:, 1 + ij] = np.concatenate([b2r[:, ij], b2r[:, ij]])
    return w1sb.astype(BF), w2sb.astype(BF), bisb


def _shard_inputs(rgb_feature, event_feature, W1, b1, W2, b2):
    rgbp = np.pad(rgb_feature, ((0, 0), (0, 0), (1, 1), (1, 1)), mode="reflect")
    # two bf16 copies of the padded slab: pixel col c at element c+2 (even
    # view, serves dj=0) and at element c+1 (odd view, serves dj=+-1).
    rgbe = np.zeros((B, C, H + 2, WE), BF)
    rgbo = np.zeros((B, C, H + 2, WE), BF)
    rgbe[:, :, :, 1:1 + W + 2] = rgbp
    rgbo[:, :, :, 0:W + 2] = rgbp
    evb = event_feature.astype(BF)
    w1sb, w2sb, bisb = _prep_consts(W1, b1, W2, b2)
    in_maps = []
    for k in range(NCORES):
        b, r0 = k // 2, SHARD_H * (k % 2)
        in_maps.append({
            "rgbe": np.ascontiguousarray(rgbe[b, :, r0:r0 + SHARD_H + 2, :]),
            "rgbo": np.ascontiguousarray(rgbo[b, :, r0:r0 + SHARD_H + 2, :]),
            "ev": np.ascontiguousarray(evb[b, :, r0:r0 + SHARD_H, :]),
            "w1": w1sb, "w2": w2sb, "bi": bisb,
        })
    return in_maps


def _run(inputs, trace=False, **trace_kwargs):
    if "nc" not in _cache:
        _cache["nc"] = _build()
    nc = _cache["nc"]
    in_maps = _shard_inputs(
        inputs["rgb_feature"].astype(np.float32),
        inputs["event_feature"].astype(np.float32),
        inputs["W1"].astype(np.float32), inputs["b1"].astype(np.float32),
        inputs["W2"].astype(np.float32), inputs["b2"].astype(np.float32))
    res = run_bass_kernel_spmd(nc, in_maps, list(range(NCORES)),
                               trace=trace, **trace_kwargs)
    full = np.empty((B, C, H, W), np.float32)
    for k in range(NCORES):
        b, r0 = k // 2, SHARD_H * (k % 2)
        full[b, :, r0:r0 + SHARD_H, :] = res.results[k]["out"]
    return full, res


def kernel(**inputs):
    full, _ = _run(inputs, trace=False)
    return full


# revision 36
# speedup vs baseline: 1.1331x; 1.1331x over previous
"""Event-RGB dynamic fusion module on 8 trn2 NeuronCores.

Per-pixel dynamic 3x3 depthwise kernels predicted from concat(rgb, event)
via two 1x1 convs + relu, applied to reflect-padded rgb.

Sharding: 8 shards = (batch b in 0..3) x (H half in {0,1}); each core gets
reflect-padded rgb slabs (two bf16 copies at element offsets 0/1 so every
3x3-shift view stays 4-byte aligned for DVE 2x mode), a bf16 event slab,
and replicated pre-laid-out bf16 weights. Fully data-parallel, no
collectives.

Pipeline per 16-row block (rows packed as [128] = 64ch x {top,bottom half}):
  mm1 (PE, K=96 via 64+32 accum)  -> h4 psum, relu+b1 on ACT -> h4 bf16
  mm2 (PE, 9 taps x [K=32,M=64] row/col-group packed) -> dk psum fp32
  apply: (dk + b2) * patch summed over taps, split across DVE/ACT/GPSIMD
"""

import os
from contextlib import ExitStack

import ml_dtypes
import numpy as np

import concourse.bass as bass
import concourse.bacc as bacc
import concourse.mybir as mybir
import concourse.tile as tile
from concourse.bass_utils import run_bass_kernel_spmd

B, C, H, W = 4, 64, 256, 256
CEV, KK, MID = 32, 3, 32
NCORES = 8
SHARD_H = 128          # rows per core
HALF = 64              # rows per half (partition-packing of pixel halves)
RBLK = 16              # rows per half per block
NBLK = HALF // RBLK    # 4
WE = 260               # padded row length (even, so shifted views stay aligned)
SUBR = 4               # rows per half per mm2/apply sub-slice (=1024 px)
NSUB = RBLK // SUBR    # 4
F32 = mybir.dt.float32
BF16 = mybir.dt.bfloat16
AOP = mybir.AluOpType
RELU = mybir.ActivationFunctionType.Relu
IDENT = mybir.ActivationFunctionType.Identity
BF = ml_dtypes.bfloat16

# apply-path assignment per tap ij: "A" = fused STT on DVE (dk read from
# PSUM); "B" = ACT copy+bias to bf16 then mul on GPSIMD; "C" = ACT copy
# then mul on DVE.
PATHS = ["A", "B", "C", "A", "B", "C", "A", "A", "C"]
# engine per tree-add (7 bf16 adds + final fp32 add): "D"=DVE, "G"=GPSIMD
ADD_ENG = ["D", "D", "G", "D", "D", "D", "D", "G"]

_cache = {}


def _build():
    nc = bacc.Bacc("TRN2", target_bir_lowering=False, debug=False)
    rgbe = nc.dram_tensor("rgbe", [C, SHARD_H + 2, WE], BF16, kind="ExternalInput").ap()
    rgbo = nc.dram_tensor("rgbo", [C, SHARD_H + 2, WE], BF16, kind="ExternalInput").ap()
    ev = nc.dram_tensor("ev", [CEV, SHARD_H, W], BF16, kind="ExternalInput").ap()
    w1 = nc.dram_tensor("w1", [128, 384], BF16, kind="ExternalInput").ap()
    w2 = nc.dram_tensor("w2", [128, 384], BF16, kind="ExternalInput").ap()
    bi = nc.dram_tensor("bi", [128, 10], F32, kind="ExternalInput").ap()
    out = nc.dram_tensor("out", [C, SHARD_H, W], F32, kind="ExternalOutput").ap()

    with tile.TileContext(nc) as tc, ExitStack() as ctx:
        _kernel(ctx, tc, rgbe, rgbo, ev, w1, w2, bi, out)
    nc.compile()
    return nc


def _kernel(ctx, tc, rgbe, rgbo, ev, w1, w2, bi, out):
    nc = tc.nc
    consts = ctx.enter_context(tc.tile_pool(name="consts", bufs=1))
    rgb_p = ctx.enter_context(tc.tile_pool(name="rgb", bufs=2))
    ev_p = ctx.enter_context(tc.tile_pool(name="evp", bufs=2))
    h4_p = ctx.enter_context(tc.tile_pool(name="h4", bufs=2))
    dkb_p = ctx.enter_context(tc.tile_pool(name="dkb", bufs=3))
    prod_p = ctx.enter_context(tc.tile_pool(name="prod", bufs=12))
    accp_p = ctx.enter_context(tc.tile_pool(name="accp", bufs=8))
    outt_p = ctx.enter_context(tc.tile_pool(name="outt", bufs=2))
    ph_p = ctx.enter_context(tc.tile_pool(name="psum_h", bufs=2, space="PSUM"))
    pdk_p = ctx.enter_context(tc.tile_pool(name="psum_dk", bufs=3, space="PSUM"))

    w1t = consts.tile([128, 384], BF16)
    nc.sync.dma_start(w1t[:], w1[:])
    w2t = consts.tile([128, 384], BF16)
    nc.sync.dma_start(w2t[:], w2[:])
    bt = consts.tile([128, 10], F32)
    nc.sync.dma_start(bt[:], bi[:])

    npx = RBLK * W           # pixels per half per block (4096)

    for t in range(NBLK):
        rge = rgb_p.tile([128, (RBLK + 2) * WE], BF16, tag="rge")
        nc.sync.dma_start(rge[0:64, :], rgbe[:, t * RBLK:t * RBLK + RBLK + 2, :])
        nc.sync.dma_start(
            rge[64:128, :], rgbe[:, HALF + t * RBLK:HALF + t * RBLK + RBLK + 2, :])
        rgo = rgb_p.tile([128, (RBLK + 2) * WE], BF16, tag="rgo")
        nc.sync.dma_start(rgo[0:64, :], rgbo[:, t * RBLK:t * RBLK + RBLK + 2, :])
        nc.sync.dma_start(
            rgo[64:128, :], rgbo[:, HALF + t * RBLK:HALF + t * RBLK + RBLK + 2, :])
        evt = ev_p.tile([128, RBLK * W], BF16)
        nc.sync.dma_start(evt[64:96, :], ev[:, t * RBLK:t * RBLK + RBLK, :])
        nc.sync.dma_start(
            evt[96:128, :], ev[:, HALF + t * RBLK:HALF + t * RBLK + RBLK, :])

        rgev = rge[:].rearrange("p (r w) -> p r w", w=WE)      # [128, 18, 260]
        rgov = rgo[:].rearrange("p (r w) -> p r w", w=WE)
        evv = evt[:].rearrange("p (r w) -> p r w", w=W)        # [128, 16, 256]

        # ---- mm1: h4[32q+m, px] = relu(b1 + W1 @ concat(rgb, ev)) x4 copies ----
        h4 = h4_p.tile([128, 2 * npx], BF16)
        for s in range(RBLK // 2):               # 512-px slices per half
            r0 = 2 * s
            ph = ph_p.tile([128, 512], F32, tag="ph")
            ph2 = ph_p.tile([128, 512], F32, tag="ph")
            # A-rgb (rows 0-63) and B-rgb (rows 64-127) stream concurrently,
            # then A-ev (64-95) and B-ev (96-127).
            nc.tensor.matmul(ph[:], w1t[0:64, 0:128],
                             rgev[0:64, r0 + 1:r0 + 3, 2:258],
                             start=True, stop=False, tile_position=(0, 0))
            nc.tensor.matmul(ph2[:], w1t[64:128, 128:256],
                             rgev[64:128, r0 + 1:r0 + 3, 2:258],
                             start=True, stop=False, tile_position=(64, 0))
            nc.tensor.matmul(ph[:], w1t[64:96, 0:128],
                             evv[64:96, r0:r0 + 2, :],
                             start=False, stop=True, tile_position=(64, 0))
            nc.tensor.matmul(ph2[:], w1t[96:128, 256:384],
                             evv[96:128, r0:r0 + 2, :],
                             start=False, stop=True, tile_position=(96, 0))
            nc.scalar.activation(h4[:, 512 * s:512 * (s + 1)], ph[:],
                                 RELU, bias=bt[:, 0:1], scale=1.0)
            nc.scalar.activation(h4[:, npx + 512 * s:npx + 512 * (s + 1)], ph2[:],
                                 RELU, bias=bt[:, 0:1], scale=1.0)

        # ---- mm2 + apply, per 4-row sub-slice (1024 px per half) ----
        # taps are processed in groups of 3 (distinct PE row-groups); the 12
        # matmuls of a group are emitted round-robin across the taps so their
        # streams run concurrently in different 32-row strips of the array.
        for s in range(NSUB):
            prods = []
            for g in range(3):
                ijs = [3 * g, 3 * g + 1, 3 * g + 2]
                dks = {ij: pdk_p.tile([128, 1024], F32, name="dk", tag="dk")
                       for ij in ijs}
                for hf in range(2):
                    for nh in range(2):
                        for ij in ijs:
                            rg, slot = ij % 4, ij // 4
                            hc0 = npx * hf + 1024 * s + 512 * nh
                            lh = w2t[32 * rg:32 * rg + 32,
                                     128 * slot + 64 * hf:128 * slot + 64 * hf + 64]
                            nc.tensor.matmul(
                                dks[ij][64 * hf:64 * hf + 64,
                                        512 * nh:512 * nh + 512],
                                lh, h4[32 * rg:32 * rg + 32, hc0:hc0 + 512],
                                start=True, stop=True,
                                tile_position=(32 * rg, 64 * hf))
                for ij in ijs:
                    di, dj = ij // 3 - 1, ij % 3 - 1
                    dk = dks[ij]
                    # patch view: dj=0 from the even-aligned slab, dj=+-1 from
                    # the odd one (keeps every bf16 view 4B-aligned).
                    if dj == 0:
                        src, base = rgev, 2
                    else:
                        src, base = rgov, 1 + dj
                    patch = src[:, SUBR * s + 1 + di:SUBR * s + 5 + di,
                                base:base + 256]
                    dkv = dk[:].rearrange("p (r w) -> p r w", w=W)
                    prod = prod_p.tile([128, 1024], BF16)
                    prodv = prod[:].rearrange("p (r w) -> p r w", w=W)
                    path = PATHS[ij]
                    if path == "A":
                        nc.vector.scalar_tensor_tensor(
                            prodv[:], dkv[:], bt[:, 1 + ij:2 + ij], patch[:],
                            op0=AOP.add, op1=AOP.mult)
                    else:
                        dkb = dkb_p.tile([128, 1024], BF16)
                        nc.scalar.activation(dkb[:], dk[:], IDENT,
                                             bias=bt[:, 1 + ij:2 + ij], scale=1.0)
                        dkbv = dkb[:].rearrange("p (r w) -> p r w", w=W)
                        eng = nc.gpsimd if path == "B" else nc.vector
                        eng.tensor_tensor(prodv[:], dkbv[:], patch[:], op=AOP.mult)
                    prods.append(prod)

            def tadd(i, a, b):
                r = accp_p.tile([128, 1024], BF16, tag="acc", name="acc")
                eng = nc.gpsimd if ADD_ENG[i] == "G" else nc.vector
                eng.tensor_tensor(r[:], a[:], b[:], op=AOP.add)
                return r

            t0 = tadd(0, prods[0], prods[1])
            t1 = tadd(1, prods[2], prods[3])
            t2 = tadd(2, prods[4], prods[5])
            t3 = tadd(3, prods[6], prods[7])
            u0 = tadd(4, t0, t1)
            u1 = tadd(5, t2, t3)
            v = tadd(6, u0, u1)
            ot = outt_p.tile([128, 1024], F32)
            eng = nc.gpsimd if ADD_ENG[7] == "G" else nc.vector
            eng.tensor_tensor(ot[:], v[:], prods[8][:], op=AOP.add)

            otv = ot[:].rearrange("p (r w) -> p r w", w=W)
            ra = t * RBLK + SUBR * s
            nc.sync.dma_start(out[:, ra:ra + SUBR, :], otv[0:64, :, :])
            nc.sync.dma_start(out[:, HALF + ra:HALF + ra + SUBR, :],
                              otv[64:128, :, :])


def _prep_consts(W1, b1, W2, b2):
    W1T = np.ascontiguousarray(W1.T)                              # [96, 32]
    W1T4 = np.tile(W1T, (1, 4))                                   # [96, 128]
    w1sb = np.zeros((128, 384), np.float32)
    w1sb[0:64, 0:128] = W1T4[0:64]          # rgb A
    w1sb[64:96, 0:128] = W1T4[64:96]        # ev A
    w1sb[64:128, 128:256] = W1T4[0:64]      # rgb B
    w1sb[96:128, 256:384] = W1T4[64:96]     # ev B

    W2r = W2.reshape(C, 9, MID)
    w2sb = np.zeros((128, 384), np.float32)
    for ij in range(9):
        rg, slot = ij % 4, ij // 4
        wij = np.ascontiguousarray(W2r[:, ij, :].T)               # [32, 64]
        w2sb[32 * rg:32 * rg + 32, 128 * slot:128 * slot + 64] = wij
        w2sb[32 * rg:32 * rg + 32, 128 * slot + 64:128 * slot + 128] = wij

    bisb = np.zeros((128, 10), np.float32)
    bisb[:, 0] = np.tile(b1, 4)
    b2r = b2.reshape(C, 9)
    for ij in range(9):
        bisb[:, 1 + ij] = np.concatenate([b2r[:, ij], b2r[:, ij]])
    return w1sb.astype(BF), w2sb.astype(BF), bisb


def _shard_inputs(rgb_feature, event_feature, W1, b1, W2, b2):
    rgbp = np.pad(rgb_feature, ((0, 0), (0, 0), (1, 1), (1, 1)), mode="reflect")
    # two bf16 copies of the padded slab: pixel col c at element c+2 (even
    # view, serves dj=0) and at element c+1 (odd view, serves dj=+-1).
    rgbe = np.zeros((B, C, H + 2, WE), BF)
    rgbo = np.zeros((B, C, H + 2, WE), BF)
    rgbe[:, :, :, 1:1 + W + 2] = rgbp
    rgbo[:, :, :, 0:W + 2] = rgbp
    evb = event_feature.astype(BF)
    w1sb, w2sb, bisb = _prep_consts(W1, b1, W2, b2)
    in_maps = []
    for k in range(NCORES):
        b, r0 = k // 2, SHARD_H * (k % 2)
        in_maps.append({
            "rgbe": np.ascontiguousarray(rgbe[b, :, r0:r0 + SHARD_H + 2, :]),
            "rgbo": np.ascontiguousarray(rgbo[b, :, r0:r0 + SHARD_H + 2, :]),
            "ev": np.ascontiguousarray(evb[b, :, r0:r0 + SHARD_H, :]),
            "w1": w1sb, "w2": w2sb, "bi": bisb,
        })
    return in_maps


def _run(inputs, trace=False, **trace_kwargs):
    if "nc" not in _cache:
        _cache["nc"] = _build()
    nc = _cache["nc"]
    in_maps = _shard_inputs(
        inputs["rgb_feature"].astype(np.float32),
        inputs["event_feature"].astype(np.float32),
        inputs["W1"].astype(np.float32), inputs["b1"].astype(np.float32),
        inputs["W2"].astype(np.float32), inputs["b2"].astype(np.float32))
    res = run_bass_kernel_spmd(nc, in_maps, list(range(NCORES)),
                               trace=trace, **trace_kwargs)
    full = np.empty((B, C, H, W), np.float32)
    for k in range(NCORES):
        b, r0 = k // 2, SHARD_H * (k % 2)
        full[b, :, r0:r0 + SHARD_H, :] = res.results[k]["out"]
    return full, res


def kernel(**inputs):
    full, _ = _run(inputs, trace=False)
    return full


# revision 37
# speedup vs baseline: 1.1367x; 1.0032x over previous
"""Event-RGB dynamic fusion module on 8 trn2 NeuronCores.

Per-pixel dynamic 3x3 depthwise kernels predicted from concat(rgb, event)
via two 1x1 convs + relu, applied to reflect-padded rgb.

Sharding: 8 shards = (batch b in 0..3) x (H half in {0,1}); each core gets
reflect-padded rgb slabs (two bf16 copies at element offsets 0/1 so every
3x3-shift view stays 4-byte aligned for DVE 2x mode), a bf16 event slab,
and replicated pre-laid-out bf16 weights. Fully data-parallel, no
collectives.

Pipeline per 16-row block (rows packed as [128] = 64ch x {top,bottom half}):
  mm1 (PE, K=96 via 64+32 accum)  -> h4 psum, relu+b1 on ACT -> h4 bf16
  mm2 (PE, 9 taps x [K=32,M=64] row/col-group packed) -> dk psum fp32
  apply: (dk + b2) * patch summed over taps, split across DVE/ACT/GPSIMD
"""

import os
from contextlib import ExitStack

import ml_dtypes
import numpy as np

import concourse.bass as bass
import concourse.bacc as bacc
import concourse.mybir as mybir
import concourse.tile as tile
from concourse.bass_utils import run_bass_kernel_spmd

B, C, H, W = 4, 64, 256, 256
CEV, KK, MID = 32, 3, 32
NCORES = 8
SHARD_H = 128          # rows per core
HALF = 64              # rows per half (partition-packing of pixel halves)
RBLK = 16              # rows per half per block
NBLK = HALF // RBLK    # 4
WE = 260               # padded row length (even, so shifted views stay aligned)
SUBR = 4               # rows per half per mm2/apply sub-slice (=1024 px)
NSUB = RBLK // SUBR    # 4
F32 = mybir.dt.float32
BF16 = mybir.dt.bfloat16
AOP = mybir.AluOpType
RELU = mybir.ActivationFunctionType.Relu
IDENT = mybir.ActivationFunctionType.Identity
BF = ml_dtypes.bfloat16

# apply-path assignment per tap ij: "A" = fused STT on DVE (dk read from
# PSUM); "B" = ACT copy+bias to bf16 then mul on GPSIMD; "C" = ACT copy
# then mul on DVE.
PATHS = ["A", "B", "C", "A", "B", "C", "A", "A", "C"]
# engine per tree-add (7 bf16 adds + final fp32 add): "D"=DVE, "G"=GPSIMD
ADD_ENG = ["D", "D", "G", "D", "D", "D", "D", "G"]

_cache = {}


def _build():
    nc = bacc.Bacc("TRN2", target_bir_lowering=False, debug=False)
    rgbe = nc.dram_tensor("rgbe", [C, SHARD_H + 2, WE], BF16, kind="ExternalInput").ap()
    rgbo = nc.dram_tensor("rgbo", [C, SHARD_H + 2, WE], BF16, kind="ExternalInput").ap()
    ev = nc.dram_tensor("ev", [CEV, SHARD_H, W], BF16, kind="ExternalInput").ap()
    w1 = nc.dram_tensor("w1", [128, 384], BF16, kind="ExternalInput").ap()
    w2 = nc.dram_tensor("w2", [128, 384], BF16, kind="ExternalInput").ap()
    bi = nc.dram_tensor("bi", [128, 10], F32, kind="ExternalInput").ap()
    out = nc.dram_tensor("out", [C, SHARD_H, W], F32, kind="ExternalOutput").ap()

    with tile.TileContext(nc) as tc, ExitStack() as ctx:
        _kernel(ctx, tc, rgbe, rgbo, ev, w1, w2, bi, out)
    nc.compile()
    return nc


def _kernel(ctx, tc, rgbe, rgbo, ev, w1, w2, bi, out):
    nc = tc.nc
    consts = ctx.enter_context(tc.tile_pool(name="consts", bufs=1))
    rgb_p = ctx.enter_context(tc.tile_pool(name="rgb", bufs=2))
    ev_p = ctx.enter_context(tc.tile_pool(name="evp", bufs=2))
    h4_p = ctx.enter_context(tc.tile_pool(name="h4", bufs=2))
    dkb_p = ctx.enter_context(tc.tile_pool(name="dkb", bufs=6))
    prod_p = ctx.enter_context(tc.tile_pool(name="prod", bufs=12))
    accp_p = ctx.enter_context(tc.tile_pool(name="accp", bufs=8))
    outt_p = ctx.enter_context(tc.tile_pool(name="outt", bufs=2))
    ph_p = ctx.enter_context(tc.tile_pool(name="psum_h", bufs=2, space="PSUM"))
    pdk_p = ctx.enter_context(tc.tile_pool(name="psum_dk", bufs=3, space="PSUM"))

    w1t = consts.tile([128, 384], BF16)
    nc.sync.dma_start(w1t[:], w1[:])
    w2t = consts.tile([128, 384], BF16)
    nc.sync.dma_start(w2t[:], w2[:])
    bt = consts.tile([128, 10], F32)
    nc.sync.dma_start(bt[:], bi[:])

    npx = RBLK * W           # pixels per half per block (4096)

    for t in range(NBLK):
        rge = rgb_p.tile([128, (RBLK + 2) * WE], BF16, tag="rge")
        nc.sync.dma_start(rge[0:64, :], rgbe[:, t * RBLK:t * RBLK + RBLK + 2, :])
        nc.sync.dma_start(
            rge[64:128, :], rgbe[:, HALF + t * RBLK:HALF + t * RBLK + RBLK + 2, :])
        rgo = rgb_p.tile([128, (RBLK + 2) * WE], BF16, tag="rgo")
        nc.sync.dma_start(rgo[0:64, :], rgbo[:, t * RBLK:t * RBLK + RBLK + 2, :])
        nc.sync.dma_start(
            rgo[64:128, :], rgbo[:, HALF + t * RBLK:HALF + t * RBLK + RBLK + 2, :])
        evt = ev_p.tile([128, RBLK * W], BF16)
        nc.sync.dma_start(evt[64:96, :], ev[:, t * RBLK:t * RBLK + RBLK, :])
        nc.sync.dma_start(
            evt[96:128, :], ev[:, HALF + t * RBLK:HALF + t * RBLK + RBLK, :])

        rgev = rge[:].rearrange("p (r w) -> p r w", w=WE)      # [128, 18, 260]
        rgov = rgo[:].rearrange("p (r w) -> p r w", w=WE)
        evv = evt[:].rearrange("p (r w) -> p r w", w=W)        # [128, 16, 256]

        # ---- mm1: h4[32q+m, px] = relu(b1 + W1 @ concat(rgb, ev)) x4 copies ----
        h4 = h4_p.tile([128, 2 * npx], BF16)
        for s in range(RBLK // 2):               # 512-px slices per half
            r0 = 2 * s
            ph = ph_p.tile([128, 512], F32, tag="ph")
            ph2 = ph_p.tile([128, 512], F32, tag="ph")
            # A-rgb (rows 0-63) and B-rgb (rows 64-127) stream concurrently,
            # then A-ev (64-95) and B-ev (96-127).
            nc.tensor.matmul(ph[:], w1t[0:64, 0:128],
                             rgev[0:64, r0 + 1:r0 + 3, 2:258],
                             start=True, stop=False, tile_position=(0, 0))
            nc.tensor.matmul(ph2[:], w1t[64:128, 128:256],
                             rgev[64:128, r0 + 1:r0 + 3, 2:258],
                             start=True, stop=False, tile_position=(64, 0))
            nc.tensor.matmul(ph[:], w1t[64:96, 0:128],
                             evv[64:96, r0:r0 + 2, :],
                             start=False, stop=True, tile_position=(64, 0))
            nc.tensor.matmul(ph2[:], w1t[96:128, 256:384],
                             evv[96:128, r0:r0 + 2, :],
                             start=False, stop=True, tile_position=(96, 0))
            nc.scalar.activation(h4[:, 512 * s:512 * (s + 1)], ph[:],
                                 RELU, bias=bt[:, 0:1], scale=1.0)
            nc.scalar.activation(h4[:, npx + 512 * s:npx + 512 * (s + 1)], ph2[:],
                                 RELU, bias=bt[:, 0:1], scale=1.0)

        # ---- mm2 + apply, per 4-row sub-slice (1024 px per half) ----
        # taps are processed in groups of 3 (distinct PE row-groups); the 12
        # matmuls of a group are emitted round-robin across the taps so their
        # streams run concurrently in different 32-row strips of the array.
        for s in range(NSUB):
            prods = []
            for g in range(3):
                ijs = [3 * g, 3 * g + 1, 3 * g + 2]
                dks = {ij: pdk_p.tile([128, 1024], F32, name="dk", tag="dk")
                       for ij in ijs}
                for hf in range(2):
                    for nh in range(2):
                        for ij in ijs:
                            rg, slot = ij % 4, ij // 4
                            hc0 = npx * hf + 1024 * s + 512 * nh
                            lh = w2t[32 * rg:32 * rg + 32,
                                     128 * slot + 64 * hf:128 * slot + 64 * hf + 64]
                            nc.tensor.matmul(
                                dks[ij][64 * hf:64 * hf + 64,
                                        512 * nh:512 * nh + 512],
                                lh, h4[32 * rg:32 * rg + 32, hc0:hc0 + 512],
                                start=True, stop=True,
                                tile_position=(32 * rg, 64 * hf))
                for ij in ijs:
                    di, dj = ij // 3 - 1, ij % 3 - 1
                    dk = dks[ij]
                    # patch view: dj=0 from the even-aligned slab, dj=+-1 from
                    # the odd one (keeps every bf16 view 4B-aligned).
                    if dj == 0:
                        src, base = rgev, 2
                    else:
                        src, base = rgov, 1 + dj
                    patch = src[:, SUBR * s + 1 + di:SUBR * s + 5 + di,
                                base:base + 256]
                    dkv = dk[:].rearrange("p (r w) -> p r w", w=W)
                    prod = prod_p.tile([128, 1024], BF16)
                    prodv = prod[:].rearrange("p (r w) -> p r w", w=W)
                    path = PATHS[ij]
                    if path == "A":
                        nc.vector.scalar_tensor_tensor(
                            prodv[:], dkv[:], bt[:, 1 + ij:2 + ij], patch[:],
                            op0=AOP.add, op1=AOP.mult)
                    else:
                        dkb = dkb_p.tile([128, 1024], BF16)
                        nc.scalar.activation(dkb[:], dk[:], IDENT,
                                             bias=bt[:, 1 + ij:2 + ij], scale=1.0)
                        dkbv = dkb[:].rearrange("p (r w) -> p r w", w=W)
                        eng = nc.gpsimd if path == "B" else nc.vector
                        eng.tensor_tensor(prodv[:], dkbv[:], patch[:], op=AOP.mult)
                    prods.append(prod)

            def tadd(i, a, b):
                r = accp_p.tile([128, 1024], BF16, tag="acc", name="acc")
                eng = nc.gpsimd if ADD_ENG[i] == "G" else nc.vector
                eng.tensor_tensor(r[:], a[:], b[:], op=AOP.add)
                return r

            t0 = tadd(0, prods[0], prods[1])
            t1 = tadd(1, prods[2], prods[3])
            t2 = tadd(2, prods[4], prods[5])
            t3 = tadd(3, prods[6], prods[7])
            u0 = tadd(4, t0, t1)
            u1 = tadd(5, t2, t3)
            v = tadd(6, u0, u1)
            ot = outt_p.tile([128, 1024], F32)
            eng = nc.gpsimd if ADD_ENG[7] == "G" else nc.vector
            eng.tensor_tensor(ot[:], v[:], prods[8][:], op=AOP.add)

            otv = ot[:].rearrange("p (r w) -> p r w", w=W)
            ra = t * RBLK + SUBR * s
            nc.sync.dma_start(out[:, ra:ra + SUBR, :], otv[0:64, :, :])
            nc.sync.dma_start(out[:, HALF + ra:HALF + ra + SUBR, :],
                              otv[64:128, :, :])


def _prep_consts(W1, b1, W2, b2):
    W1T = np.ascontiguousarray(W1.T)                              # [96, 32]
    W1T4 = np.tile(W1T, (1, 4))                                   # [96, 128]
    w1sb = np.zeros((128, 384), np.float32)
    w1sb[0:64, 0:128] = W1T4[0:64]          # rgb A
    w1sb[64:96, 0:128] = W1T4[64:96]        # ev A
    w1sb[64:128, 128:256] = W1T4[0:64]      # rgb B
    w1sb[96:128, 256:384] = W1T4[64:96]     # ev B

    W2r = W2.reshape(C, 9, MID)
    w2sb = np.zeros((128, 384), np.float32)
    for ij in range(9):
        rg, slot = ij % 4, ij // 4
        wij = np.ascontiguousarray(W2r[:, ij, :].T)               # [32, 64]
        w2sb[32 * rg:32 * rg + 32, 128 * slot:128 * slot + 64] = wij
        w2sb[32 * rg:32 * rg + 32, 128 * slot + 64:128 * slot + 128] = wij

    bisb = np.zeros((128, 10), np.float32)
    bisb[:, 0] = np.tile(b1, 4)
    b2r = b2.reshape(C, 9)
    for ij in range(9):
        bisb[:, 1 + ij] = np.concatenate([b2r[:, ij], b2r[:, ij]])
    return w1sb.astype(BF), w2sb.astype(BF), bisb


def _shard_inputs(rgb_feature, event_feature, W1, b1, W2, b2):
    rgbp = np.pad(rgb_feature, ((0, 0), (0, 0), (1, 1), (1, 1)), mode="reflect")
    # two bf16 copies of the padded slab: pixel col c at element c+2 (even
    # view, serves dj=0) and at element c+1 (odd view, serves dj=+-1).
    rgbe = np.zeros((B, C, H + 2, WE), BF)
    rgbo = np.zeros((B, C, H + 2, WE), BF)
    rgbe[:, :, :, 1:1 + W + 2] = rgbp
    rgbo[:, :, :, 0:W + 2] = rgbp
    evb = event_feature.astype(BF)
    w1sb, w2sb, bisb = _prep_consts(W1, b1, W2, b2)
    in_maps = []
    for k in range(NCORES):
        b, r0 = k // 2, SHARD_H * (k % 2)
        in_maps.append({
            "rgbe": np.ascontiguousarray(rgbe[b, :, r0:r0 + SHARD_H + 2, :]),
            "rgbo": np.ascontiguousarray(rgbo[b, :, r0:r0 + SHARD_H + 2, :]),
            "ev": np.ascontiguousarray(evb[b, :, r0:r0 + SHARD_H, :]),
            "w1": w1sb, "w2": w2sb, "bi": bisb,
        })
    return in_maps


def _run(inputs, trace=False, **trace_kwargs):
    if "nc" not in _cache:
        _cache["nc"] = _build()
    nc = _cache["nc"]
    in_maps = _shard_inputs(
        inputs["rgb_feature"].astype(np.float32),
        inputs["event_feature"].astype(np.float32),
        inputs["W1"].astype(np.float32), inputs["b1"].astype(np.float32),
        inputs["W2"].astype(np.float32), inputs["b2"].astype(np.float32))
    res = run_bass_kernel_spmd(nc, in_maps, list(range(NCORES)),
                               trace=trace, **trace_kwargs)
    full = np.empty((B, C, H, W), np.float32)
    for k in range(NCORES):
        b, r0 = k // 2, SHARD_H * (k % 2)
        full[b, :, r0:r0 + SHARD_H, :] = res.results[k]["out"]
    return full, res


def kernel(**inputs):
    full, _ = _run(inputs, trace=False)
    return full


# revision 38
# speedup vs baseline: 1.1379x; 1.0010x over previous
"""Event-RGB dynamic fusion module on 8 trn2 NeuronCores.

Per-pixel dynamic 3x3 depthwise kernels predicted from concat(rgb, event)
via two 1x1 convs + relu, applied to reflect-padded rgb.

Sharding: 8 shards = (batch b in 0..3) x (H half in {0,1}); each core gets
reflect-padded rgb slabs (two bf16 copies at element offsets 0/1 so every
3x3-shift view stays 4-byte aligned for DVE 2x mode), a bf16 event slab,
and replicated pre-laid-out bf16 weights. Fully data-parallel, no
collectives.

Pipeline per 16-row block (rows packed as [128] = 64ch x {top,bottom half}):
  mm1 (PE, K=96 via 64+32 accum)  -> h4 psum, relu+b1 on ACT -> h4 bf16
  mm2 (PE, 9 taps x [K=32,M=64] row/col-group packed) -> dk psum fp32
  apply: (dk + b2) * patch summed over taps, split across DVE/ACT/GPSIMD
"""

import os
from contextlib import ExitStack

import ml_dtypes
import numpy as np

import concourse.bass as bass
import concourse.bacc as bacc
import concourse.mybir as mybir
import concourse.tile as tile
from concourse.bass_utils import run_bass_kernel_spmd

B, C, H, W = 4, 64, 256, 256
CEV, KK, MID = 32, 3, 32
NCORES = 8
SHARD_H = 128          # rows per core
HALF = 64              # rows per half (partition-packing of pixel halves)
RBLK = 16              # rows per half per block
NBLK = HALF // RBLK    # 4
WE = 260               # padded row length (even, so shifted views stay aligned)
SUBR = 4               # rows per half per mm2/apply sub-slice (=1024 px)
NSUB = RBLK // SUBR    # 4
F32 = mybir.dt.float32
BF16 = mybir.dt.bfloat16
AOP = mybir.AluOpType
RELU = mybir.ActivationFunctionType.Relu
IDENT = mybir.ActivationFunctionType.Identity
BF = ml_dtypes.bfloat16

# apply-path assignment per tap ij: "A" = fused STT on DVE (dk read from
# PSUM); "B" = ACT copy+bias to bf16 then mul on GPSIMD; "C" = ACT copy
# then mul on DVE.
PATHS = ["A", "B", "C", "A", "B", "C", "A", "A", "C"]
# engine per tree-add (7 bf16 adds + final fp32 add): "D"=DVE, "G"=GPSIMD
ADD_ENG = ["D", "D", "G", "D", "D", "D", "D", "G"]

_cache = {}


def _build():
    nc = bacc.Bacc("TRN2", target_bir_lowering=False, debug=False)
    rgbe = nc.dram_tensor("rgbe", [C, SHARD_H + 2, WE], BF16, kind="ExternalInput").ap()
    rgbo = nc.dram_tensor("rgbo", [C, SHARD_H + 2, WE], BF16, kind="ExternalInput").ap()
    ev = nc.dram_tensor("ev", [CEV, SHARD_H, W], BF16, kind="ExternalInput").ap()
    w1 = nc.dram_tensor("w1", [128, 384], BF16, kind="ExternalInput").ap()
    w2 = nc.dram_tensor("w2", [128, 384], BF16, kind="ExternalInput").ap()
    bi = nc.dram_tensor("bi", [128, 10], F32, kind="ExternalInput").ap()
    out = nc.dram_tensor("out", [C, SHARD_H, W], F32, kind="ExternalOutput").ap()

    with tile.TileContext(nc) as tc, ExitStack() as ctx:
        _kernel(ctx, tc, rgbe, rgbo, ev, w1, w2, bi, out)
    nc.compile()
    return nc


def _kernel(ctx, tc, rgbe, rgbo, ev, w1, w2, bi, out):
    nc = tc.nc
    consts = ctx.enter_context(tc.tile_pool(name="consts", bufs=1))
    rgb_p = ctx.enter_context(tc.tile_pool(name="rgb", bufs=2))
    ev_p = ctx.enter_context(tc.tile_pool(name="evp", bufs=2))
    h4_p = ctx.enter_context(tc.tile_pool(name="h4", bufs=2))
    dkb_p = ctx.enter_context(tc.tile_pool(name="dkb", bufs=6))
    prod_p = ctx.enter_context(tc.tile_pool(name="prod", bufs=12))
    accp_p = ctx.enter_context(tc.tile_pool(name="accp", bufs=8))
    outt_p = ctx.enter_context(tc.tile_pool(name="outt", bufs=4))
    ph_p = ctx.enter_context(tc.tile_pool(name="psum_h", bufs=2, space="PSUM"))
    pdk_p = ctx.enter_context(tc.tile_pool(name="psum_dk", bufs=3, space="PSUM"))

    w1t = consts.tile([128, 384], BF16)
    nc.sync.dma_start(w1t[:], w1[:])
    w2t = consts.tile([128, 384], BF16)
    nc.sync.dma_start(w2t[:], w2[:])
    bt = consts.tile([128, 10], F32)
    nc.sync.dma_start(bt[:], bi[:])

    npx = RBLK * W           # pixels per half per block (4096)

    for t in range(NBLK):
        rge = rgb_p.tile([128, (RBLK + 2) * WE], BF16, tag="rge")
        nc.sync.dma_start(rge[0:64, :], rgbe[:, t * RBLK:t * RBLK + RBLK + 2, :])
        nc.sync.dma_start(
            rge[64:128, :], rgbe[:, HALF + t * RBLK:HALF + t * RBLK + RBLK + 2, :])
        rgo = rgb_p.tile([128, (RBLK + 2) * WE], BF16, tag="rgo")
        nc.sync.dma_start(rgo[0:64, :], rgbo[:, t * RBLK:t * RBLK + RBLK + 2, :])
        nc.sync.dma_start(
            rgo[64:128, :], rgbo[:, HALF + t * RBLK:HALF + t * RBLK + RBLK + 2, :])
        evt = ev_p.tile([128, RBLK * W], BF16)
        nc.sync.dma_start(evt[64:96, :], ev[:, t * RBLK:t * RBLK + RBLK, :])
        nc.sync.dma_start(
            evt[96:128, :], ev[:, HALF + t * RBLK:HALF + t * RBLK + RBLK, :])

        rgev = rge[:].rearrange("p (r w) -> p r w", w=WE)      # [128, 18, 260]
        rgov = rgo[:].rearrange("p (r w) -> p r w", w=WE)
        evv = evt[:].rearrange("p (r w) -> p r w", w=W)        # [128, 16, 256]

        # ---- mm1: h4[32q+m, px] = relu(b1 + W1 @ concat(rgb, ev)) x4 copies ----
        h4 = h4_p.tile([128, 2 * npx], BF16)
        for s in range(RBLK // 2):               # 512-px slices per half
            r0 = 2 * s
            ph = ph_p.tile([128, 512], F32, tag="ph")
            ph2 = ph_p.tile([128, 512], F32, tag="ph")
            # A-rgb (rows 0-63) and B-rgb (rows 64-127) stream concurrently,
            # then A-ev (64-95) and B-ev (96-127).
            nc.tensor.matmul(ph[:], w1t[0:64, 0:128],
                             rgev[0:64, r0 + 1:r0 + 3, 2:258],
                             start=True, stop=False, tile_position=(0, 0))
            nc.tensor.matmul(ph2[:], w1t[64:128, 128:256],
                             rgev[64:128, r0 + 1:r0 + 3, 2:258],
                             start=True, stop=False, tile_position=(64, 0))
            nc.tensor.matmul(ph[:], w1t[64:96, 0:128],
                             evv[64:96, r0:r0 + 2, :],
                             start=False, stop=True, tile_position=(64, 0))
            nc.tensor.matmul(ph2[:], w1t[96:128, 256:384],
                             evv[96:128, r0:r0 + 2, :],
                             start=False, stop=True, tile_position=(96, 0))
            nc.scalar.activation(h4[:, 512 * s:512 * (s + 1)], ph[:],
                                 RELU, bias=bt[:, 0:1], scale=1.0)
            nc.scalar.activation(h4[:, npx + 512 * s:npx + 512 * (s + 1)], ph2[:],
                                 RELU, bias=bt[:, 0:1], scale=1.0)

        # ---- mm2 + apply, per 4-row sub-slice (1024 px per half) ----
        # taps are processed in groups of 3 (distinct PE row-groups); the 12
        # matmuls of a group are emitted round-robin across the taps so their
        # streams run concurrently in different 32-row strips of the array.
        for s in range(NSUB):
            prods = []
            for g in range(3):
                ijs = [3 * g, 3 * g + 1, 3 * g + 2]
                dks = {ij: pdk_p.tile([128, 1024], F32, name="dk", tag="dk")
                       for ij in ijs}
                for hf in range(2):
                    for nh in range(2):
                        for ij in ijs:
                            rg, slot = ij % 4, ij // 4
                            hc0 = npx * hf + 1024 * s + 512 * nh
                            lh = w2t[32 * rg:32 * rg + 32,
                                     128 * slot + 64 * hf:128 * slot + 64 * hf + 64]
                            nc.tensor.matmul(
                                dks[ij][64 * hf:64 * hf + 64,
                                        512 * nh:512 * nh + 512],
                                lh, h4[32 * rg:32 * rg + 32, hc0:hc0 + 512],
                                start=True, stop=True,
                                tile_position=(32 * rg, 64 * hf))
                for ij in ijs:
                    di, dj = ij // 3 - 1, ij % 3 - 1
                    dk = dks[ij]
                    # patch view: dj=0 from the even-aligned slab, dj=+-1 from
                    # the odd one (keeps every bf16 view 4B-aligned).
                    if dj == 0:
                        src, base = rgev, 2
                    else:
                        src, base = rgov, 1 + dj
                    patch = src[:, SUBR * s + 1 + di:SUBR * s + 5 + di,
                                base:base + 256]
                    dkv = dk[:].rearrange("p (r w) -> p r w", w=W)
                    prod = prod_p.tile([128, 1024], BF16)
                    prodv = prod[:].rearrange("p (r w) -> p r w", w=W)
                    path = PATHS[ij]
                    if path == "A":
                        nc.vector.scalar_tensor_tensor(
                            prodv[:], dkv[:], bt[:, 1 + ij:2 + ij], patch[:],
                            op0=AOP.add, op1=AOP.mult)
                    else:
                        dkb = dkb_p.tile([128, 1024], BF16)
                        nc.scalar.activation(dkb[:], dk[:], IDENT,
                                             bias=bt[:, 1 + ij:2 + ij], scale=1.0)
                        dkbv = dkb[:].rearrange("p (r w) -> p r w", w=W)
                        eng = nc.gpsimd if path == "B" else nc.vector
                        eng.tensor_tensor(prodv[:], dkbv[:], patch[:], op=AOP.mult)
                    prods.append(prod)

            def tadd(i, a, b):
                r = accp_p.tile([128, 1024], BF16, tag="acc", name="acc")
                eng = nc.gpsimd if ADD_ENG[i] == "G" else nc.vector
                eng.tensor_tensor(r[:], a[:], b[:], op=AOP.add)
                return r

            t0 = tadd(0, prods[0], prods[1])
            t1 = tadd(1, prods[2], prods[3])
            t2 = tadd(2, prods[4], prods[5])
            t3 = tadd(3, prods[6], prods[7])
            u0 = tadd(4, t0, t1)
            u1 = tadd(5, t2, t3)
            v = tadd(6, u0, u1)
            ot = outt_p.tile([128, 1024], F32)
            eng = nc.gpsimd if ADD_ENG[7] == "G" else nc.vector
            eng.tensor_tensor(ot[:], v[:], prods[8][:], op=AOP.add)

            otv = ot[:].rearrange("p (r w) -> p r w", w=W)
            ra = t * RBLK + SUBR * s
            nc.sync.dma_start(out[:, ra:ra + SUBR, :], otv[0:64, :, :])
            nc.sync.dma_start(out[:, HALF + ra:HALF + ra + SUBR, :],
                              otv[64:128, :, :])


def _prep_consts(W1, b1, W2, b2):
    W1T = np.ascontiguousarray(W1.T)                              # [96, 32]
    W1T4 = np.tile(W1T, (1, 4))                                   # [96, 128]
    w1sb = np.zeros((128, 384), np.float32)
    w1sb[0:64, 0:128] = W1T4[0:64]          # rgb A
    w1sb[64:96, 0:128] = W1T4[64:96]        # ev A
    w1sb[64:128, 128:256] = W1T4[0:64]      # rgb B
    w1sb[96:128, 256:384] = W1T4[64:96]     # ev B

    W2r = W2.reshape(C, 9, MID)
    w2sb = np.zeros((128, 384), np.float32)
    for ij in range(9):
        rg, slot = ij % 4, ij // 4
        wij = np.ascontiguousarray(W2r[:, ij, :].T)               # [32, 64]
        w2sb[32 * rg:32 * rg + 32, 128 * slot:128 * slot + 64] = wij
        w2sb[32 * rg:32 * rg + 32, 128 * slot + 64:128 * slot + 128] = wij

    bisb = np.zeros((128, 10), np.float32)
    bisb[:, 0] = np.tile(b1, 4)
    b2r = b2.reshape(C, 9)
    for ij in range(9):
        bisb[:, 1 + ij] = np.concatenate([b2r[:, ij], b2r[:, ij]])
    return w1sb.astype(BF), w2sb.astype(BF), bisb


def _shard_inputs(rgb_feature, event_feature, W1, b1, W2, b2):
    rgbp = np.pad(rgb_feature, ((0, 0), (0, 0), (1, 1), (1, 1)), mode="reflect")
    # two bf16 copies of the padded slab: pixel col c at element c+2 (even
    # view, serves dj=0) and at element c+1 (odd view, serves dj=+-1).
    rgbe = np.zeros((B, C, H + 2, WE), BF)
    rgbo = np.zeros((B, C, H + 2, WE), BF)
    rgbe[:, :, :, 1:1 + W + 2] = rgbp
    rgbo[:, :, :, 0:W + 2] = rgbp
    evb = event_feature.astype(BF)
    w1sb, w2sb, bisb = _prep_consts(W1, b1, W2, b2)
    in_maps = []
    for k in range(NCORES):
        b, r0 = k // 2, SHARD_H * (k % 2)
        in_maps.append({
            "rgbe": np.ascontiguousarray(rgbe[b, :, r0:r0 + SHARD_H + 2, :]),
            "rgbo": np.ascontiguousarray(rgbo[b, :, r0:r0 + SHARD_H + 2, :]),
            "ev": np.ascontiguousarray(evb[b, :, r0:r0 + SHARD_H, :]),
            "w1": w1sb, "w2": w2sb, "bi": bisb,
        })
    return in_maps


def _run(inputs, trace=False, **trace_kwargs):
    if "nc" not in _cache:
        _cache["nc"] = _build()
    nc = _cache["nc"]
    in_maps = _shard_inputs(
        inputs["rgb_feature"].astype(np.float32),
        inputs["event_feature"].astype(np.float32),
        inputs["W1"].astype(np.float32), inputs["b1"].astype(np.float32),
        inputs["W2"].astype(np.float32), inputs["b2"].astype(np.float32))
    res = run_bass_kernel_spmd(nc, in_maps, list(range(NCORES)),
                               trace=trace, **trace_kwargs)
    full = np.empty((B, C, H, W), np.float32)
    for k in range(NCORES):
        b, r0 = k // 2, SHARD_H * (k % 2)
        full[b, :, r0:r0 + SHARD_H, :] = res.results[k]["out"]
    return full, res


def kernel(**inputs):
    full, _ = _run(inputs, trace=False)
    return full
